# revision 65
# baseline (speedup 1.0000x reference)
"""Trainium2 Bass kernel: single attention head (B=8, S=2048, E=1024, H=64).

Sharding: data-parallel over batch -- each of the 8 NeuronCores computes one
batch element's full attention. No collectives; every HBM byte read once.

v9 design (duplicated-weight projections, chunk-major q, 512-col kv blocks):
  - Inputs cast to fp16 HOST-side; HWDGE (sync) block DMAs with contiguous
    per-partition lines. Half the HBM traffic of the f32 original.
  - q projection: the stationary is [Wq | Wq] ([128, 128] per chunk, FWL
    eligible), so ONE matmul per (chunk, 512-col segment) yields q^T in
    BOTH SBUF partition halves -- no column tiling, 8x fewer LDWEIGHTS
    (each weight load costs ~110-180ns of PE time since every matmul
    re-loads its stationary). Chunk-major over resident xq halves.
  - k/v stream in four 512-col blocks; per block one column-tiled k||v
    projection pass (k in partition half (jb%2), v in the other), feeding
    row-tiled scores: tiles from even blocks run in array rows 0-63
    concurrently with odd-block tiles in rows 64-127 into different PSUM
    banks. Scores at N=512.
  - exp on ScalarE over [128, 1024] slabs (32 calls, ~36us total -- the
    global throughput floor). AV matmuls ([v|1]^T @ exp(S^T) into a
    [65, 2048] PSUM accumulator) are queued and used as always-ready
    filler between score groups so the in-order PE FIFO stays dense and
    the HAM clock gate holds 2.4 GHz.
  - Scores stay transposed (keys on partitions); softmax rowsums ride the
    ones column of the AV stationary; bk cancels in softmax; bq/bv fold
    into projection evacuations.
  - Finalize: PE transposes 128-col chunks, VectorE reciprocal + scale,
    batched f32 DMA out.

PSUM: 2 x 2-bank rotating slots (projections + score slabs + finalize
transposes) + 4 banks AV accumulator = 8 banks exactly.
"""

import numpy as np

import concourse.bass as bass  # noqa: F401  (engine namespaces live on nc)
import concourse.mybir as mybir
import concourse.tile as tile
from concourse import bacc
from concourse.bass_utils import run_bass_kernel_spmd
from concourse.masks import make_identity

B, S, E, H = 8, 2048, 1024, 64
EC = E // 128    # contraction chunks (128 partitions each)
KB = 512         # kv block columns
NKB = S // KB    # 4 kv blocks
NT = S // 128    # key tiles
F16 = mybir.dt.float16
F32 = mybir.dt.float32

_CACHE = {}


def _build_nc():
    nc = bacc.Bacc(None)
    xq = nc.declare_dram_parameter("xq", [128, 2, EC, S // 2], F16, isOutput=False)
    xk = nc.declare_dram_parameter("xk", [128, NKB, EC, KB], F16, isOutput=False)
    xv = nc.declare_dram_parameter("xv", [128, NKB, EC, KB], F16, isOutput=False)
    wqd = nc.declare_dram_parameter("wqd", [128, EC, 128], F16, isOutput=False)
    wk = nc.declare_dram_parameter("wk", [128, EC, H], F16, isOutput=False)
    wv = nc.declare_dram_parameter("wv", [128, EC, H], F16, isOutput=False)
    bq = nc.declare_dram_parameter("bq", [128, 1], F32, isOutput=False)
    bv = nc.declare_dram_parameter("bv", [128, 1], F32, isOutput=False)
    out = nc.declare_dram_parameter("out", [S, H], F32, isOutput=True)

    Exp = mybir.ActivationFunctionType.Exp

    with tile.TileContext(nc) as tc:
        with tc.tile_pool(name="const", bufs=1) as const, \
             tc.tile_pool(name="xkp", bufs=4) as xkp, \
             tc.tile_pool(name="xvp", bufs=4) as xvp, \
             tc.tile_pool(name="ptp", bufs=10) as ptp, \
             tc.tile_pool(name="vtp", bufs=2) as vtp, \
             tc.tile_pool(name="p5sb", bufs=2) as p5sb, \
             tc.tile_pool(name="psp", bufs=2, space="PSUM") as psp, \
             tc.tile_pool(name="oap", bufs=1, space="PSUM") as oap:

            # ---- constants ----
            # weights/biases go on the scalar HWDGE ring so their dispatch
            # cost doesn't head-of-line block the input stream on sync
            wqd_t = const.tile([128, EC, 128], F16, name="wqd_t")
            nc.scalar.dma_start(out=wqd_t[:], in_=wqd[:])
            wk_t = const.tile([128, EC, H], F16, name="wk_t")
            nc.scalar.dma_start(out=wk_t[:], in_=wk[:])
            wv_t = const.tile([128, EC, H], F16, name="wv_t")
            nc.scalar.dma_start(out=wv_t[:], in_=wv[:])
            bq_t = const.tile([128, 1], F32, name="bq_t")
            nc.scalar.dma_start(out=bq_t[:], in_=bq[:])
            bv_t = const.tile([128, 1], F32, name="bv_t")
            nc.scalar.dma_start(out=bv_t[:], in_=bv[:])

            qt = const.tile([128, S], F16, name="qt")     # q^T in BOTH halves
            kt = const.tile([128, S], F16, name="kt")     # k^T: half (jb%2)
            xqt = const.tile([128, EC, S], F16, name="xqt")
            vaug = const.tile([128, NT, 80], F16, name="vaug")
            oasb = const.tile([65, S], F16, name="oasb")
            ident = const.tile([128, 128], F16, name="ident")
            osb_all = const.tile([128, NT, H], F32, name="osb_all")

            make_identity(nc, ident[:])
            nc.vector.memset(vaug[:, :, 64], 1.0)

            oa = oap.tile([65, S], F32, name="oa")        # AV accumulator

            def slot(name):
                return psp.tile([128, 1024], F32, tag="ps", name=name)

            # ---- input DMAs (sync HWDGE FIFO) ----
            xkts, xvts = [], []

            def fetch(which, jb):
                if which == "k":
                    xt = xkp.tile([128, EC, KB], F16, tag="xk", name=f"xkt{jb}")
                    nc.sync.dma_start(out=xt[:], in_=xk[:, jb])
                    xkts.append(xt)
                else:
                    xt = xvp.tile([128, EC, KB], F16, tag="xv", name=f"xvt{jb}")
                    nc.sync.dma_start(out=xt[:], in_=xv[:, jb])
                    xvts.append(xt)

            nc.sync.dma_start(out=xqt[:, :, 0:1024], in_=xq[:, 0])
            fetch("k", 0)
            fetch("v", 0)
            fetch("k", 1)
            fetch("v", 1)
            nc.sync.dma_start(out=xqt[:, :, 1024:2048], in_=xq[:, 1])
            fetch("k", 2)
            fetch("v", 2)
            fetch("k", 3)
            fetch("v", 3)

            # ---- PE warm-keeper: spans the preamble + xq DMA head so the
            # HAM clock gate is at 2.4 GHz when the first projection runs.
            wslot = slot("warm")
            for _ in range(66):
                nc.tensor.matmul(
                    wslot[0:128, 0:128], ident[:], ident[:],
                    start=True, stop=True, skip_group_check=True)

            # ---- AV queue: always-ready filler matmuls ----
            pts = [None] * NT           # per-tile exp(S^T) SBUF tiles
            av_ready = []
            av_bank_count = [0] * 4

            def emit_av(n):
                while n > 0 and av_ready:
                    t, qh = av_ready.pop(0)
                    for sg in range(2):
                        seg = 2 * qh + sg
                        cnt = av_bank_count[seg]
                        nc.tensor.matmul(
                            oa[:, seg * 512:(seg + 1) * 512],
                            vaug[:, t, 0:65],
                            pts[t][:, seg * 512:(seg + 1) * 512],
                            start=(cnt == 0), stop=(cnt == NT - 1),
                            skip_group_check=True)
                        av_bank_count[seg] = cnt + 1
                    n -= 1

            def qproj(qh):
                ps = slot(f"pq{qh}")
                for c in range(EC):
                    for sg in range(2):
                        nc.tensor.matmul(
                            ps[:, sg * 512:(sg + 1) * 512],
                            wqd_t[:, c, :],
                            xqt[:, c, qh * 1024 + sg * 512:
                                qh * 1024 + (sg + 1) * 512],
                            start=(c == 0), stop=(c == EC - 1),
                            skip_group_check=True)
                nc.vector.tensor_scalar_add(
                    qt[:, qh * 1024:(qh + 1) * 1024], ps[:], bq_t[:])

            def kvproj(jb):
                kh = (jb % 2) * 64
                vh = 64 - kh
                ps = slot(f"pkv{jb}")
                for c in range(EC):
                    nc.tensor.matmul(
                        ps[kh:kh + 64, 0:KB], wk_t[:, c, :], xkts[jb][:, c, :],
                        start=(c == 0), stop=(c == EC - 1),
                        skip_group_check=True)
                    nc.tensor.matmul(
                        ps[vh:vh + 64, 0:KB], wv_t[:, c, :], xvts[jb][:, c, :],
                        start=(c == 0), stop=(c == EC - 1),
                        skip_group_check=True)
                vtb = vtp.tile([128, KB], F16, tag="vt", name=f"vtb{jb}")
                nc.vector.tensor_scalar_add(
                    vtb[vh:vh + 64, :], ps[vh:vh + 64, 0:KB], bv_t[vh:vh + 64])
                nc.sync.dma_start_transpose(
                    vaug[:, 4 * jb:4 * jb + 4, 0:64], vtb[vh:vh + 64, :])
                nc.vector.tensor_copy(
                    kt[kh:kh + 64, jb * KB:(jb + 1) * KB], ps[kh:kh + 64, 0:KB])

            def score_slabs(cells):
                """Scores + exp for a list of (tile, qh) cells. Consecutive
                cells with opposite row-group parity run concurrently on the
                PE (row tiling) since their slabs sit in different banks."""
                for t, qh in cells:
                    if pts[t] is None:
                        pts[t] = ptp.tile([128, S], F16, tag="pt", name=f"pt{t}")
                mms, exps = [], []
                for t, qh in cells:
                    g = ((t // 4) % 2) * 64
                    sl = slot(f"s{t}_{qh}")
                    for seg in range(2):
                        cs = slice(qh * 1024 + seg * 512,
                                   qh * 1024 + (seg + 1) * 512)
                        mms.append((sl, seg, g, t, cs))
                    exps.append((t, qh, sl))
                # interleave the two cells' matmuls seg-by-seg for pairing
                if len(cells) == 2:
                    mms = [mms[0], mms[2], mms[1], mms[3]]
                for sl, seg, g, t, cs in mms:
                    nc.tensor.matmul(
                        sl[:, seg * 512:(seg + 1) * 512],
                        kt[g:g + 64, t * 128:(t + 1) * 128], qt[g:g + 64, cs],
                        start=True, stop=True, skip_group_check=True)
                for t, qh, sl in exps:
                    nc.scalar.activation(
                        pts[t][:, qh * 1024:(qh + 1) * 1024], sl[:],
                        Exp, scale=0.125)
                    av_ready.append((t, qh))
                emit_av(len(cells))

            # ---- schedule ----
            qproj(0)
            kvproj(0)
            for t in range(4):                    # unpaired, earliest exp
                score_slabs([(t, 0)])
            kvproj(1)
            qproj(1)
            for t in range(4, 8):                 # unpaired: bridges xqh1 DMA
                score_slabs([(t, 0)])
            for pi in range(4):                   # (h0, h1) pairs, q half 1
                score_slabs([(pi, 1), (pi + 4, 1)])
            kvproj(2)
            kvproj(3)
            for qh in range(2):
                for pi in range(4):
                    score_slabs([(8 + pi, qh), (12 + pi, qh)])
            # ---- finalize: transpose, normalize, store ----
            # 4 transposes per PSUM slot, one batched reciprocal per chunk
            out_r = out[:].rearrange("(t p) h -> p t h", p=128)

            def finalize_chunk(cq):
                nc.vector.tensor_copy(
                    oasb[:, cq * 512:(cq + 1) * 512],
                    oa[:, cq * 512:(cq + 1) * 512])
                trs = psp.tile([128, 4, 66], F16, tag="ps", name=f"trs{cq}")
                for jj in range(4):
                    j = cq * 4 + jj
                    nc.tensor.transpose(
                        trs[:, jj, 0:65], oasb[:, j * 128:(j + 1) * 128],
                        ident[0:65, 0:65])
                rc = p5sb.tile([128, 4], F32, tag="rc", name=f"rc{cq}")
                nc.vector.reciprocal(rc[:], trs[:, :, 64])
                for jj in range(4):
                    j = cq * 4 + jj
                    nc.vector.tensor_scalar(
                        osb_all[:, j, :], trs[:, jj, 0:64], rc[:, jj:jj + 1],
                        None, op0=mybir.AluOpType.mult)
                nc.scalar.dma_start(
                    out=out_r[:, cq * 4:(cq + 1) * 4, :],
                    in_=osb_all[:, cq * 4:(cq + 1) * 4, :])

            # flush q-half-0 AVs first: oa banks 0/1 stop, so chunks 0/1
            # finalize while the q-half-1 AV matmuls still accumulate 2/3
            qh0_left = [cq for cq in av_ready if cq[1] == 0]
            qh1_left = [cq for cq in av_ready if cq[1] == 1]
            av_ready[:] = qh0_left
            emit_av(len(qh0_left))
            finalize_chunk(0)
            finalize_chunk(1)
            av_ready[:] = qh1_left
            emit_av(len(qh1_left))
            finalize_chunk(2)
            finalize_chunk(3)

    nc.finalize()
    return nc


def get_nc():
    if "nc" not in _CACHE:
        _CACHE["nc"] = _build_nc()
    return _CACHE["nc"]


def _stage_x(x, nblk, cb):
    # [S, E] f32 -> [128, nblk, EC, cb] f16 with [p, b, c, s] = x[b*cb+s, c*128+p]
    xt = np.ascontiguousarray(x.T.astype(np.float16))          # [E, S]
    xt = xt.reshape(EC, 128, nblk, cb).transpose(1, 2, 0, 3)   # [p, b, c, s]
    return np.ascontiguousarray(xt)


def make_in_maps(inputs):
    q = np.asarray(inputs["query"], np.float32)
    k = np.asarray(inputs["key_"], np.float32)
    v = np.asarray(inputs["value"], np.float32)
    wq_h = np.asarray(inputs["Wq"], np.float32).astype(np.float16)
    wqd_h = np.concatenate([wq_h, wq_h], axis=1)                # [E, 128]
    wqd_s = np.ascontiguousarray(
        wqd_h.reshape(EC, 128, 128).transpose(1, 0, 2))         # [128, EC, 128]
    wmats = {}
    for nm, key in (("wk", "Wk"), ("wv", "Wv")):
        w = np.asarray(inputs[key], np.float32).astype(np.float16)
        wmats[nm] = np.ascontiguousarray(
            w.reshape(EC, 128, H).transpose(1, 0, 2))           # [128, EC, H]
    bq = np.asarray(inputs["bq"], np.float32).reshape(H, 1)
    bv = np.asarray(inputs["bv"], np.float32).reshape(H, 1)
    bq_d = np.ascontiguousarray(np.tile(bq, (2, 1)))            # [128, 1]
    bv_d = np.ascontiguousarray(np.tile(bv, (2, 1)))
    in_maps = []
    for b in range(B):
        in_maps.append({
            "xq": _stage_x(q[b], 2, S // 2),
            "xk": _stage_x(k[b], NKB, KB),
            "xv": _stage_x(v[b], NKB, KB),
            "wqd": wqd_s, "wk": wmats["wk"], "wv": wmats["wv"],
            "bq": bq_d, "bv": bv_d,
        })
    return in_maps


def kernel(**inputs):
    nc = get_nc()
    in_maps = make_in_maps(inputs)
    res = run_bass_kernel_spmd(nc, in_maps, list(range(B)))
    return np.stack([res.results[b]["out"] for b in range(B)], axis=0)
